# revision 3
# baseline (speedup 1.0000x reference)
"""Causal multi-head self-attention (RoPE, V-uses-Q-projection bug preserved)
as a Bass/Tile kernel for 8 Trainium2 NeuronCores — v2.

Sharding: core c -> batch b = c//4, head-group g = c%4 (4 heads of 16).
Each core computes its 4 heads' attention for its batch and a partial
output projection; partials are summed per batch on the host.

v2 changes vs v1 (180.9us):
  - bf16 matmul datapath (f32 PSUM accumulation, f32 biases/exp/divide):
    no narrow-matmul fp32r penalty, 1.0 c/r transposes, half the DMA bytes.
  - cos/sin RoPE tables precomputed on host (kills the on-device
    Cody-Waite + Sin chain that serialized the prologue).
  - batched DMAs ([128,8,512] per tile-group) split across SP/Act HWDGE
    queues; out DMAs per t-tile on the Act queue.
  - oproj bias folded into the PSUM->SBUF copy on DVE (no PE bias matmuls).
  - attention q-chunked at 1024 with AV accumulation into per-512-chunk
    PSUM tiles; output projection for a 512-row chunk is emitted as soon
    as all 4 heads finish it, interleaved into the attention stream.
  - causal mask applied multiplicatively post-exp on GpSimd (SBUF only),
    freeing DVE; exp table preloaded by a dummy activation at t=0.
"""

import os
from contextlib import ExitStack

import numpy as np

import concourse.bass as bass
import concourse.mybir as mybir
import concourse.tile as tile
from concourse import bacc
from concourse.bass import ds, ts
from concourse.masks import make_identity

F32 = mybir.dt.float32
BF16 = mybir.dt.bfloat16
AF = mybir.ActivationFunctionType
ALU = mybir.AluOpType

B, T, D, H, DK = 2, 2048, 1024, 16, 64
THETA = 10000.0
NCORE, GPB = 8, 4          # cores; head-groups per batch
HPC = H // GPB             # heads per core = 4
OC = HPC * DK              # per-core projected features = 256
NT = T // 128              # 16 t-tiles
NDC = D // 128             # 8 contraction chunks
NG, GT = 4, 4              # t-groups; t-tiles per group


def build_kernel():
    nc = bacc.Bacc("TRN2", target_bir_lowering=False, debug=False)

    xT = nc.dram_tensor("xT", [D, T], BF16, kind="ExternalInput")
    wqk = nc.dram_tensor("wqk", [D, 2 * OC], BF16, kind="ExternalInput")
    wo = nc.dram_tensor("wo", [OC, D], BF16, kind="ExternalInput")
    bqk = nc.dram_tensor("bqk", [1, 2 * OC], F32, kind="ExternalInput")
    bo = nc.dram_tensor("bo", [1, D], F32, kind="ExternalInput")
    cosT = nc.dram_tensor("cosT", [128, NT * DK], F32, kind="ExternalInput")
    sinT = nc.dram_tensor("sinT", [128, NT * (DK // 2)], F32, kind="ExternalInput")
    maskt = nc.dram_tensor("maskt", [128, 128], BF16, kind="ExternalInput")
    out = nc.dram_tensor("out", [T, D], F32, kind="ExternalOutput")

    with tile.TileContext(nc) as tc, ExitStack() as top:
        consts = top.enter_context(tc.tile_pool(name="consts", bufs=1))
        wpool = top.enter_context(tc.tile_pool(name="weights", bufs=1))
        vk = top.enter_context(tc.tile_pool(name="vk", bufs=1))
        qtkt = top.enter_context(tc.tile_pool(name="qtkt", bufs=1))
        heads_pool = top.enter_context(tc.tile_pool(name="heads", bufs=1))

        # ---- weights first on the SP queue (gate the first matmuls) ----
        wqk_sb = wpool.tile([128, NDC, 2 * OC], BF16)
        wqk_v = wqk.ap().rearrange("(dc p) c -> p dc c", p=128)
        nc.sync.dma_start(wqk_sb[:, 0:4, :], wqk_v[:, 0:4, :])
        nc.sync.dma_start(wqk_sb[:, 4:8, :], wqk_v[:, 4:8, :])

        # local compute while DMAs fly
        ident = consts.tile([128, 128], BF16)
        make_identity(nc, ident[:])
        v_sb = vk.tile([128, NT, HPC, 2 * DK], BF16)
        nc.gpsimd.memset(v_sb[:, :, :, DK:], 1.0)
        dume = consts.tile([1, 8], F32)
        nc.vector.memset(dume[:], 0.0)
        dume2 = consts.tile([1, 8], F32)
        nc.scalar.activation(dume2[:], dume[:], AF.Exp)  # preload Exp table

        # ---- consts on the Act queue ----
        maskt_sb = consts.tile([128, 128], BF16)
        nc.scalar.dma_start(maskt_sb[:], maskt.ap())
        bqk_rep = consts.tile([128, 2 * OC], F32)
        nc.scalar.dma_start(bqk_rep[:], bqk.ap().to_broadcast((128, 2 * OC)))
        bo_rep = consts.tile([128, D], F32)
        nc.scalar.dma_start(bo_rep[:], bo.ap().to_broadcast((128, D)))
        cos_sb = consts.tile([128, NT, DK], F32)
        nc.scalar.dma_start(
            cos_sb[:], cosT.ap().rearrange("p (j i) -> p j i", j=NT)
        )
        sin_sb = consts.tile([128, NT, DK // 2], F32)
        nc.scalar.dma_start(
            sin_sb[:], sinT.ap().rearrange("p (j i) -> p j i", j=NT)
        )
        wo_sb = wpool.tile([128, 2, D], BF16)
        nc.scalar.dma_start(wo_sb[:], wo.ap().rearrange("(jc p) d -> p jc d", p=128))

        qt_sb = [
            qtkt.tile([128, T // 2], BF16, tag=f"qt{i}", name=f"qt_sb{i}")
            for i in range(4)
        ]
        kt_sb = [
            qtkt.tile([128, T // 2], BF16, tag=f"kt{i}", name=f"kt_sb{i}")
            for i in range(4)
        ]
        heads_t = [
            heads_pool.tile([128, T // 2], BF16, tag=f"ht{i}", name=f"heads_t{i}")
            for i in range(4)
        ]

        def rope(eng, src, dst, pool, tg, pfx):
            """One t-group of rotary embedding; src/dst views [p, GT, HPC, DK]."""
            m = pool.tile([128, GT, HPC, DK], BF16, tag=f"{pfx}m", name=f"{pfx}m{tg}")
            s = pool.tile(
                [128, GT, HPC, DK // 2], BF16, tag=f"{pfx}s", name=f"{pfx}s{tg}"
            )
            tsl = slice(tg * GT, (tg + 1) * GT)
            cos_bc = cos_sb[:, tsl].unsqueeze(2).to_broadcast((128, GT, HPC, DK))
            sin_bc = sin_sb[:, tsl].unsqueeze(2).to_broadcast((128, GT, HPC, DK // 2))
            x1 = src[:, :, :, 0 : DK // 2]
            x2 = src[:, :, :, DK // 2 : DK]
            eng.tensor_tensor(m[:], src, cos_bc, ALU.mult)
            eng.tensor_tensor(s[:], x2, sin_bc, ALU.mult)
            eng.tensor_tensor(
                dst[:, :, :, 0 : DK // 2], m[:, :, :, 0 : DK // 2], s[:], ALU.subtract
            )
            eng.tensor_tensor(s[:], x1, sin_bc, ALU.mult)
            eng.tensor_tensor(
                dst[:, :, :, DK // 2 : DK], m[:, :, :, DK // 2 : DK], s[:], ALU.add
            )

        # ---- phase P: projection + rope + transpose, per t-group ----
        with tc.tile_pool(name="xt", bufs=2) as xtp, \
             tc.tile_pool(name="ropep", bufs=2) as ropep, \
             tc.tile_pool(name="ps_proj", bufs=int(os.environ.get("PSP_BUFS", "6")), space="PSUM") as psp, \
             tc.tile_pool(name="ps_tp", bufs=int(os.environ.get("TP_BUFS", "2")), space="PSUM") as pst:
            xT_v = xT.ap().rearrange("(dc p) t -> p dc t", p=128)
            xts = []

            def issue_xt(tg, eng):
                t_ = xtp.tile([128, NDC, 512], BF16, tag="xt", name=f"xt{tg}")
                eng.dma_start(t_[:], xT_v[:, :, ts(tg, 512)])
                xts.append(t_)

            issue_xt(0, nc.scalar)
            issue_xt(1, nc.sync)
            for tg in range(NG):
                if tg + 2 < NG:
                    issue_xt(tg + 2, nc.scalar if tg % 2 else nc.sync)
                xt = xts[tg]
                k_sb = ropep.tile([128, GT, OC], BF16, tag="k_sb", name=f"ks{tg}")
                for tl in range(GT):
                    t = GT * tg + tl
                    ps = psp.tile([128, 2 * OC], F32, tag="pproj")
                    for dc in range(NDC):
                        nc.tensor.matmul(
                            ps[:],
                            xt[:, dc, ts(tl, 128)],
                            wqk_sb[:, dc, :],
                            start=(dc == 0),
                            stop=(dc == NDC - 1),
                        )
                    nc.vector.tensor_tensor(
                        v_sb[:, t, :, 0:DK],
                        ps[:, 0:OC].rearrange("p (h f) -> p h f", h=HPC),
                        bqk_rep[:, 0:OC].rearrange("p (h f) -> p h f", h=HPC),
                        ALU.add,
                    )
                    nc.vector.tensor_add(
                        k_sb[:, tl, :], ps[:, OC : 2 * OC], bqk_rep[:, OC : 2 * OC]
                    )
                q_rope = ropep.tile(
                    [128, GT, HPC, DK], BF16, tag="q_rope", name=f"qr{tg}"
                )
                rope(nc.vector, v_sb[:, ts(tg, GT), :, 0:DK], q_rope[:], ropep, tg, "q")
                k_rope = ropep.tile(
                    [128, GT, HPC, DK], BF16, tag="k_rope", name=f"kr{tg}"
                )
                k_view = k_sb[:].rearrange("p t (h f) -> p t h f", h=HPC)
                rope(nc.gpsimd, k_view, k_rope[:], ropep, tg, "k")
                for srcv, dst in ((q_rope, qt_sb), (k_rope, kt_sb)):
                    sv = srcv[:].rearrange("p t h f -> p t (h f)")
                    for oc in range(2):
                        tp = pst.tile([128, 512], BF16, tag="tp")
                        for tl in range(GT):
                            nc.tensor.transpose(
                                tp[:, ts(tl, 128)], sv[:, tl, ts(oc, 128)], ident[:]
                            )
                        nc.scalar.copy(dst[oc * 2 + tg // 2][:, ts(tg % 2, 512)], tp[:])

        # ---- phase A: attention with interleaved output projection ----
        scale = float(1.0 / np.sqrt(DK))
        with tc.tile_pool(name="expp", bufs=int(os.environ.get("ET_BUFS", "6"))) as expp, \
             tc.tile_pool(name="divp", bufs=int(os.environ.get("DIV_BUFS", "2"))) as divp, \
             tc.tile_pool(name="outp", bufs=int(os.environ.get("OUTP_BUFS", "4"))) as outp, \
             tc.tile_pool(name="ps_sc", bufs=int(os.environ.get("SC_BUFS", "2")), space="PSUM") as pssc, \
             tc.tile_pool(name="ps_o2", bufs=int(os.environ.get("O2_BUFS", "2")), space="PSUM") as pso2, \
             tc.tile_pool(name="ps_po", bufs=int(os.environ.get("OP_BUFS", "2")), space="PSUM") as psop:

            def divide_chunk(h, c2, c, o2):
                # o2 rows 0..63 = head features, rows 64..127 = softmax denom.
                oc, ro = h // 2, 64 * (h % 2)
                rec_f = divp.tile([128, 512], F32, tag="rec_f", name=f"rf{c2}{h}{c}")
                nc.vector.reciprocal(rec_f[ds(64, 64), :], o2[ds(64, 64), :])
                rec_lo = divp.tile([64, 512], F32, tag="rec_lo", name=f"rl{c2}{h}{c}")
                nc.sync.dma_start(rec_lo[:], rec_f[ds(64, 64), :])
                dstcols = ts(c - 2 * c2, 512)
                if ro == 0:
                    nc.vector.tensor_tensor(
                        heads_t[oc * 2 + c2][ds(0, 64), dstcols],
                        o2[ds(0, 64), :],
                        rec_lo[:],
                        ALU.mult,
                    )
                else:
                    stage = divp.tile([64, 512], BF16, tag="stage", name=f"st{c2}{h}{c}")
                    nc.vector.tensor_tensor(
                        stage[:], o2[ds(0, 64), :], rec_lo[:], ALU.mult
                    )
                    nc.sync.dma_start(
                        heads_t[oc * 2 + c2][ds(ro, 64), dstcols], stage[:]
                    )

            def oproj_chunk(c):
                c2 = c // 2
                for tl in range(4):
                    t = 4 * c + tl
                    ot = outp.tile([128, D], F32, tag="ot", name=f"ot{t}")
                    for ic in range(2):
                        po = psop.tile([128, 512], F32, tag="po")
                        for oc in range(2):
                            nc.tensor.matmul(
                                po[:],
                                heads_t[oc * 2 + c2][:, ds(128 * (t - 8 * c2), 128)],
                                wo_sb[:, oc, ts(ic, 512)],
                                start=(oc == 0),
                                stop=(oc == 1),
                            )
                        nc.vector.tensor_add(
                            ot[:, ts(ic, 512)], po[:], bo_rep[:, ts(ic, 512)]
                        )
                    nc.scalar.dma_start(out.ap()[ts(t, 128), :], ot[:])

            for c2 in range(2):
                q0 = 1024 * c2
                nkt = 8 * (c2 + 1)
                for h in range(HPC):
                    oc, ro = h // 2, 64 * (h % 2)
                    qt_h = qt_sb[oc * 2 + c2][ds(ro, 64), :]
                    o2c = {
                        c: pso2.tile([128, 512], F32, tag="o2", name=f"o2_{c2}{h}{c}")
                        for c in (2 * c2, 2 * c2 + 1)
                    }
                    for kt in range(nkt):
                        qs = max(q0, 128 * kt)
                        cw = q0 + 1024 - qs
                        sc = pssc.tile([128, 1024], F32, tag="sc")
                        for n5 in range((cw + 511) // 512):
                            ns = qs + 512 * n5
                            nw = min(512, q0 + 1024 - ns)
                            nc.tensor.matmul(
                                sc[:, ds(512 * n5, nw)],
                                kt_sb[oc * 2 + kt // 8][ds(ro, 64), ts(kt % 8, 128)],
                                qt_h[:, ds(ns - q0, nw)],
                                start=True,
                                stop=True,
                            )
                        et = expp.tile([128, 1024], BF16, tag="et")
                        nc.scalar.activation(
                            et[:, ds(0, cw)], sc[:, ds(0, cw)], AF.Exp, scale=scale
                        )
                        if qs == 128 * kt:  # zero the above-diagonal region
                            nc.gpsimd.tensor_tensor(
                                et[:, 0:128], et[:, 0:128], maskt_sb[:], ALU.mult
                            )
                        for c in (2 * c2, 2 * c2 + 1):
                            ce = 512 * (c + 1)
                            if ce <= qs:
                                continue
                            ns = max(qs, 512 * c)
                            nw = ce - ns
                            nc.tensor.matmul(
                                o2c[c][:, ds(ns - 512 * c, nw)],
                                v_sb[:, kt, h, :],
                                et[:, ds(ns - qs, nw)],
                                start=(kt == 0),
                                stop=(kt == 4 * c + 3),
                            )
                        for c in (2 * c2, 2 * c2 + 1):
                            if kt == 4 * c + 3:
                                divide_chunk(h, c2, c, o2c[c])
                                if h == HPC - 1:
                                    oproj_chunk(c)

    nc.compile()
    return nc


_NC_CACHE = None


def _get_nc():
    global _NC_CACHE
    if _NC_CACHE is None:
        _NC_CACHE = build_kernel()
    return _NC_CACHE


_PERM = np.concatenate([np.arange(0, DK, 2), np.arange(1, DK, 2)])


def make_in_maps(in_features, token_positions, Wq, bq, Wk, bk, Wo, bo):
    import ml_dtypes

    BF = ml_dtypes.bfloat16
    x = np.ascontiguousarray(np.asarray(in_features, dtype=np.float32))
    pos = np.asarray(token_positions, dtype=np.float32)
    Wq = np.asarray(Wq, dtype=np.float32)
    bq = np.asarray(bq, dtype=np.float32)
    Wk = np.asarray(Wk, dtype=np.float32)
    bk = np.asarray(bk, dtype=np.float32)
    Wo = np.asarray(Wo, dtype=np.float32)
    bo = np.asarray(bo, dtype=np.float32)

    inv = (1.0 / THETA ** (np.arange(0, DK, 2, dtype=np.float32) / DK)).astype(
        np.float32
    )
    ang = pos[:, None] * inv[None, :]  # [T, 32]
    cos = np.cos(ang).astype(np.float32)
    sin = np.sin(ang).astype(np.float32)
    # table layout: [p, j, i] with token t = 128*j + p
    cosT = cos.reshape(NT, 128, DK // 2).transpose(1, 0, 2)
    cosT = np.ascontiguousarray(
        np.concatenate([cosT, cosT], axis=2).reshape(128, NT * DK)
    )
    sinT = np.ascontiguousarray(
        sin.reshape(NT, 128, DK // 2).transpose(1, 0, 2).reshape(128, NT * (DK // 2))
    )
    ii = np.arange(128)
    maskt = (ii[None, :] >= ii[:, None]).astype(BF)  # [k, q]: keep q >= k

    in_maps = []
    for c in range(NCORE):
        b, g = c // GPB, c % GPB
        cols = np.concatenate([DK * (HPC * g + hh) + _PERM for hh in range(HPC)])
        in_maps.append(
            {
                "xT": np.ascontiguousarray(x[b].T).astype(BF),
                "wqk": np.ascontiguousarray(
                    np.concatenate([Wq[cols].T, Wk[cols].T], axis=1)
                ).astype(BF),
                "wo": np.ascontiguousarray(Wo[:, cols].T).astype(BF),
                "bqk": np.ascontiguousarray(
                    np.concatenate([bq[cols], bk[cols]])[None, :]
                ),
                "bo": np.ascontiguousarray(
                    (bo if g == 0 else np.zeros_like(bo))[None, :]
                ),
                "cosT": cosT,
                "sinT": sinT,
                "maskt": maskt,
            }
        )
    return in_maps


def kernel(in_features, token_positions, Wq, bq, Wk, bk, Wv=None, bv=None, Wo=None, bo=None):
    from concourse import bass_utils

    nc = _get_nc()
    in_maps = make_in_maps(in_features, token_positions, Wq, bq, Wk, bk, Wo, bo)
    res = bass_utils.run_bass_kernel_spmd(
        nc,
        in_maps,
        core_ids=list(range(NCORE)),
        trace=bool(int(os.environ.get("KERNEL_TRACE", "0"))),
    )
    outs = [res.results[c]["out"] for c in range(NCORE)]
    full = np.stack(
        [np.sum(outs[b * GPB : (b + 1) * GPB], axis=0) for b in range(B)]
    ).astype(np.float32)
    kernel.last_results = res
    return full


# revision 10
# speedup vs baseline: 1.0601x; 1.0601x over previous
"""Causal multi-head self-attention (RoPE, V-uses-Q-projection bug preserved)
as a Bass/Tile kernel for 8 Trainium2 NeuronCores — v3.

Sharding: core c -> batch b = c//4, head-group g = c%4 (4 heads of 16).
Each core computes its 4 heads' attention for its batch and a partial
output projection; partials are summed per batch on the host.

Pipeline (single PSUM layout: sc 2x[128,1024]f32 | o2 2x[128,512]f32 |
pp 2x[128,512] shared ring for proj/transpose/oproj/den-broadcast):
  P : projection+rope+transpose for t-groups 0,1 (q/k tiles 0..7)
  A0: attention q-chunk [0,1024) — Act(exp)-bound, so projection+rope+
      transpose of t-groups 2,3 are interleaved as PE filler
  A1: attention q-chunk [1024,2048) — output projection of finished
      512-row chunks interleaved as PE filler
  division: AV uses V'=[V|1] (even heads) / [1|V] (odd heads) so the
      softmax denominator lands on the opposite 64 partitions; one row is
      copied to SBUF, PE-broadcast to all partitions, then a lane-aligned
      elementwise divide writes heads_t directly (no partition-shift DMA).
Out DMAs ride the SP queue (data-dependent DMAs park the issuing
sequencer, so Act only issues dep-free input DMAs at t=0).
"""

import os
from contextlib import ExitStack

import numpy as np

import concourse.bass as bass
import concourse.mybir as mybir
import concourse.tile as tile
from concourse import bacc
from concourse.bass import ds, ts
from concourse.masks import make_identity

F32 = mybir.dt.float32
BF16 = mybir.dt.bfloat16
AF = mybir.ActivationFunctionType
ALU = mybir.AluOpType

B, T, D, H, DK = 2, 2048, 1024, 16, 64
THETA = 10000.0
NCORE, GPB = 8, 4          # cores; head-groups per batch
HPC = H // GPB             # heads per core = 4
OC = HPC * DK              # per-core projected features = 256
NT = T // 128              # 16 t-tiles
NDC = D // 128             # 8 contraction chunks
NG, GT = 4, 4              # t-groups; t-tiles per group


def build_kernel():
    nc = bacc.Bacc("TRN2", target_bir_lowering=False, debug=False)

    xT = nc.dram_tensor("xT", [D, T], BF16, kind="ExternalInput")
    wqk = nc.dram_tensor("wqk", [D, 2 * OC], BF16, kind="ExternalInput")
    wo = nc.dram_tensor("wo", [OC, D], BF16, kind="ExternalInput")
    bqk = nc.dram_tensor("bqk", [1, 2 * OC], F32, kind="ExternalInput")
    bo = nc.dram_tensor("bo", [1, D], F32, kind="ExternalInput")
    cosT = nc.dram_tensor("cosT", [128, NT * DK], F32, kind="ExternalInput")
    sinT = nc.dram_tensor("sinT", [128, NT * (DK // 2)], F32, kind="ExternalInput")
    maskt = nc.dram_tensor("maskt", [128, 128], BF16, kind="ExternalInput")
    out = nc.dram_tensor("out", [T, D], F32, kind="ExternalOutput")

    with tile.TileContext(nc) as tc, ExitStack() as top:
        consts = top.enter_context(tc.tile_pool(name="consts", bufs=1))
        wpool = top.enter_context(tc.tile_pool(name="weights", bufs=1))
        vk = top.enter_context(tc.tile_pool(name="vk", bufs=1))
        qtkt = top.enter_context(tc.tile_pool(name="qtkt", bufs=1))
        heads_pool = top.enter_context(tc.tile_pool(name="heads", bufs=1))

        # ---- input DMAs: SP gets what gates the first matmuls ----
        wqk_sb = wpool.tile([128, NDC, 2 * OC], BF16)
        wqk_v = wqk.ap().rearrange("(dc p) c -> p dc c", p=128)
        nc.sync.dma_start(wqk_sb[:, 0:4, :], wqk_v[:, 0:4, :])
        nc.sync.dma_start(wqk_sb[:, 4:8, :], wqk_v[:, 4:8, :])

        xT_v = xT.ap().rearrange("(dc p) t -> p dc t", p=128)
        xtp = top.enter_context(tc.tile_pool(name="xt", bufs=2))
        xts = []

        def issue_xt(tg, eng):
            t_ = xtp.tile([128, NDC, 512], BF16, tag="xt", name=f"xt{tg}")
            eng.dma_start(t_[:], xT_v[:, :, ts(tg, 512)])
            xts.append(t_)

        issue_xt(0, nc.scalar)
        issue_xt(1, nc.sync)

        # local compute while DMAs fly
        ident = consts.tile([128, 128], BF16)
        make_identity(nc, ident[:])
        ones_col = consts.tile([1, 128], BF16)
        nc.gpsimd.memset(ones_col[:], 1.0)
        v_sb = vk.tile([128, NT, HPC, 2 * DK], BF16)
        for h in range(HPC):  # ones on the side opposite the V features
            sl = slice(DK, 2 * DK) if h % 2 == 0 else slice(0, DK)
            nc.gpsimd.memset(v_sb[:, :, h, sl], 1.0)
        dume = consts.tile([1, 8], F32)
        nc.vector.memset(dume[:], 0.0)
        dume2 = consts.tile([1, 8], F32)
        nc.scalar.activation(dume2[:], dume[:], AF.Exp)  # preload Exp table

        # remaining inputs on the Act queue (no data deps -> never parks)
        bqk_rep = consts.tile([128, 2 * OC], F32)
        nc.scalar.dma_start(bqk_rep[:], bqk.ap().to_broadcast((128, 2 * OC)))
        cos_sb = consts.tile([128, NT, DK], F32)
        nc.scalar.dma_start(cos_sb[:], cosT.ap().rearrange("p (j i) -> p j i", j=NT))
        sin_sb = consts.tile([128, NT, DK // 2], F32)
        nc.scalar.dma_start(sin_sb[:], sinT.ap().rearrange("p (j i) -> p j i", j=NT))
        issue_xt(2, nc.scalar)
        maskt_sb = consts.tile([128, 128], BF16)
        nc.scalar.dma_start(maskt_sb[:], maskt.ap())
        issue_xt(3, nc.scalar)
        wo_sb = wpool.tile([128, 2, D], BF16)
        nc.scalar.dma_start(wo_sb[:], wo.ap().rearrange("(jc p) d -> p jc d", p=128))
        bo_rep = consts.tile([128, D], F32)
        nc.scalar.dma_start(bo_rep[:], bo.ap().to_broadcast((128, D)))

        qt_sb = [
            qtkt.tile([128, T // 2], BF16, tag=f"qt{i}", name=f"qt_sb{i}")
            for i in range(4)
        ]
        kt_sb = [
            qtkt.tile([128, T // 2], BF16, tag=f"kt{i}", name=f"kt_sb{i}")
            for i in range(4)
        ]
        heads_t = [
            heads_pool.tile([128, T // 2], BF16, tag=f"ht{i}", name=f"heads_t{i}")
            for i in range(4)
        ]

        with tc.tile_pool(name="expp", bufs=int(os.environ.get("ET_BUFS", "6"))) as expp, \
             tc.tile_pool(name="divp", bufs=int(os.environ.get("DIV_BUFS", "3"))) as divp, \
             tc.tile_pool(name="outp", bufs=int(os.environ.get("OUTP_BUFS", "4"))) as outp, \
             tc.tile_pool(name="ropep", bufs=2) as ropep, \
             tc.tile_pool(name="psA", bufs=2, space="PSUM") as psA:

            k_sbs = {}

            def proj_tile(tg, tl):
                xt = xts[tg]
                t = GT * tg + tl
                ps = psA.tile([128, 2 * OC], F32, tag="pp", name=f"ps{t}")
                for dc in range(NDC):
                    nc.tensor.matmul(
                        ps[:],
                        xt[:, dc, ts(tl, 128)],
                        wqk_sb[:, dc, :],
                        start=(dc == 0),
                        stop=(dc == NDC - 1),
                    )
                # V = Q + bias, in [V|1] (even h) / [1|V] (odd h) layout
                vv = v_sb[:, t].rearrange("p (pr a) f -> p pr (a f)", pr=HPC // 2)
                pv = ps[:, 0:OC].rearrange("p (pr c) -> p pr c", pr=HPC // 2)
                bv = bqk_rep[:, 0:OC].rearrange("p (pr c) -> p pr c", pr=HPC // 2)
                nc.vector.tensor_tensor(
                    vv[:, :, 0:DK], pv[:, :, 0:DK], bv[:, :, 0:DK], ALU.add
                )
                nc.vector.tensor_tensor(
                    vv[:, :, 3 * DK : 4 * DK],
                    pv[:, :, DK : 2 * DK],
                    bv[:, :, DK : 2 * DK],
                    ALU.add,
                )
                nc.vector.tensor_add(
                    k_sbs[tg][:, tl, :], ps[:, OC : 2 * OC], bqk_rep[:, OC : 2 * OC]
                )

            def rope(eng, src, dst, tg, pfx):
                """src/dst views [p, GT, n, DK]; cos/sin broadcast over dim 2."""
                n = src.shape[2]
                m = ropep.tile([128, GT, n, DK], BF16, tag=f"{pfx}m", name=f"{pfx}m{tg}")
                s = ropep.tile(
                    [128, GT, n, DK // 2], BF16, tag=f"{pfx}s", name=f"{pfx}s{tg}"
                )
                tsl = slice(tg * GT, (tg + 1) * GT)
                cos_bc = cos_sb[:, tsl].unsqueeze(2).to_broadcast((128, GT, n, DK))
                sin_bc = sin_sb[:, tsl].unsqueeze(2).to_broadcast((128, GT, n, DK // 2))
                x1 = src[:, :, :, 0 : DK // 2]
                x2 = src[:, :, :, DK // 2 : DK]
                eng.tensor_tensor(m[:], src, cos_bc, ALU.mult)
                eng.tensor_tensor(s[:], x2, sin_bc, ALU.mult)
                eng.tensor_tensor(
                    dst[:, :, :, 0 : DK // 2], m[:, :, :, 0 : DK // 2], s[:], ALU.subtract
                )
                eng.tensor_tensor(s[:], x1, sin_bc, ALU.mult)
                eng.tensor_tensor(
                    dst[:, :, :, DK // 2 : DK], m[:, :, :, DK // 2 : DK], s[:], ALU.add
                )

            def rope_q(tg):
                q_rope = ropep.tile(
                    [128, GT, HPC, DK], BF16, tag="q_rope", name=f"qr{tg}"
                )
                qv = q_rope[:].rearrange("p t (pr two) f -> p t pr (two f)", two=2)
                vv = v_sb[:, ts(tg, GT)].rearrange("p t (pr a) f -> p t pr (a f)", pr=2)
                rope(nc.vector, vv[:, :, :, 0:DK], qv[:, :, :, 0:DK], tg, "q")
                rope(nc.vector, vv[:, :, :, 3 * DK : 4 * DK], qv[:, :, :, DK : 2 * DK], tg, "q2")
                return q_rope

            def rope_k(tg):
                k_rope = ropep.tile(
                    [128, GT, HPC, DK], BF16, tag="k_rope", name=f"kr{tg}"
                )
                k_view = k_sbs[tg][:].rearrange("p t (h f) -> p t h f", h=HPC)
                rope(nc.gpsimd, k_view, k_rope[:], tg, "k")
                return k_rope

            def tgroup(srcv, dst, tg, oc, cpeng):
                sv = srcv[:].rearrange("p t h f -> p t (h f)")
                tp = psA.tile([128, 512], BF16, tag="pp", name=f"tp{tg}{oc}")
                for tl in range(GT):
                    nc.tensor.transpose(
                        tp[:, ts(tl, 128)], sv[:, tl, ts(oc, 128)], ident[:]
                    )
                d_ = dst[oc * 2 + tg // 2][:, ts(tg % 2, 512)]
                if cpeng is nc.scalar:
                    cpeng.copy(d_, tp[:])
                else:
                    cpeng.tensor_copy(d_, tp[:])

            def full_tg(tg, cpeng):
                k_sbs[tg] = ropep.tile([128, GT, OC], BF16, tag="k_sb", name=f"ks{tg}")
                for tl in range(GT):
                    proj_tile(tg, tl)
                q_rope = rope_q(tg)
                k_rope = rope_k(tg)
                for oc in range(2):
                    tgroup(q_rope, qt_sb, tg, oc, cpeng)
                for oc in range(2):
                    tgroup(k_rope, kt_sb, tg, oc, cpeng)

            fillers = []

            def drain(n=1):
                for _ in range(min(n, len(fillers))):
                    fillers.pop(0)()

            def queue_tg(tg):
                def mk_ksb():
                    k_sbs[tg] = ropep.tile(
                        [128, GT, OC], BF16, tag="k_sb", name=f"ks{tg}"
                    )

                fillers.append(mk_ksb)
                for tl in range(GT):
                    fillers.append(lambda tg=tg, tl=tl: proj_tile(tg, tl))
                holder = {}

                def do_ropes(tg=tg):
                    holder["q"] = rope_q(tg)
                    holder["k"] = rope_k(tg)

                fillers.append(do_ropes)
                for src, dst in (("q", qt_sb), ("k", kt_sb)):
                    for oc in range(2):
                        fillers.append(
                            lambda src=src, dst=dst, tg=tg, oc=oc: tgroup(
                                holder[src], dst, tg, oc,
                                nc.scalar if src == "q" else nc.vector,
                            )
                        )

            def divide_chunk(h, c2, c, o2):
                # parity flip puts head h's features at rows ro..ro+63 and the
                # replicated denominator on the opposite 64 partitions
                oc, ro = h // 2, 64 * (h % 2)
                rec = divp.tile([128, 512], F32, tag="rec", name=f"rc{c2}{h}{c}")
                nc.vector.reciprocal(rec[ds(64 - ro, 64), :], o2[ds(64 - ro, 64), :])
                rsh = divp.tile([128, 512], F32, tag="rsh", name=f"rs{c2}{h}{c}")
                deng = nc.scalar if (c2 == 1 and c == 3) else nc.sync
                deng.dma_start(rsh[ds(ro, 64), :], rec[ds(64 - ro, 64), :])
                nc.vector.tensor_tensor(
                    heads_t[oc * 2 + c2][ds(ro, 64), ts(c - 2 * c2, 512)],
                    o2[ds(ro, 64), :],
                    rsh[ds(ro, 64), :],
                    ALU.mult,
                )

            def oproj_chunk_units(c):
                c2 = c // 2
                units = []
                for tl in range(4):
                    t = 4 * c + tl
                    ot = outp.tile([128, D], F32, tag="ot", name=f"ot{t}")
                    for ic in range(2):
                        def unit(t=t, ic=ic, ot=ot, c2=c2):
                            po = psA.tile([128, 512], F32, tag="pp", name=f"po{t}{ic}")
                            for jc in range(2):
                                nc.tensor.matmul(
                                    po[:],
                                    heads_t[jc * 2 + c2][:, ds(128 * (t - 8 * c2), 128)],
                                    wo_sb[:, jc, ts(ic, 512)],
                                    start=(jc == 0),
                                    stop=(jc == 1),
                                )
                            nc.vector.tensor_add(
                                ot[:, ts(ic, 512)], po[:], bo_rep[:, ts(ic, 512)]
                            )
                            if ic == 1:
                                nc.sync.dma_start(out.ap()[ts(t, 128), :], ot[:])
                        units.append(unit)
                return units

            scale = float(1.0 / np.sqrt(DK))

            def attention(c2):
                q0 = 1024 * c2
                nkt = 8 * (c2 + 1)
                for h in range(HPC):
                    oc, ro = h // 2, 64 * (h % 2)
                    qt_h = qt_sb[oc * 2 + c2][ds(ro, 64), :]
                    o2c = {
                        c: psA.tile([128, 512], F32, tag="o2", name=f"o2_{c2}{h}{c}")
                        for c in (2 * c2, 2 * c2 + 1)
                    }
                    for kt in range(nkt):
                        qs = max(q0, 128 * kt)
                        cw = q0 + 1024 - qs
                        sc = psA.tile([128, 1024], F32, tag="sc", name=f"sc{c2}{h}{kt}")
                        for n5 in range((cw + 511) // 512):
                            ns = qs + 512 * n5
                            nw = min(512, q0 + 1024 - ns)
                            nc.tensor.matmul(
                                sc[:, ds(512 * n5, nw)],
                                kt_sb[oc * 2 + kt // 8][ds(ro, 64), ts(kt % 8, 128)],
                                qt_h[:, ds(ns - q0, nw)],
                                start=True,
                                stop=True,
                            )
                        et = expp.tile([128, 1024], BF16, tag="et", name=f"et{c2}{h}{kt}")
                        nc.scalar.activation(
                            et[:, ds(0, cw)], sc[:, ds(0, cw)], AF.Exp, scale=scale
                        )
                        if qs == 128 * kt:  # zero the above-diagonal region
                            nc.gpsimd.tensor_tensor(
                                et[:, 0:128], et[:, 0:128], maskt_sb[:], ALU.mult
                            )
                        for c in (2 * c2, 2 * c2 + 1):
                            ce = 512 * (c + 1)
                            if ce <= qs:
                                continue
                            ns = max(qs, 512 * c)
                            nw = ce - ns
                            nc.tensor.matmul(
                                o2c[c][:, ds(ns - 512 * c, nw)],
                                v_sb[:, kt, h, :],
                                et[:, ds(ns - qs, nw)],
                                start=(kt == 0),
                                stop=(kt == 4 * c + 3),
                            )
                        for c in (2 * c2, 2 * c2 + 1):
                            if kt == 4 * c + 3:
                                divide_chunk(h, c2, c, o2c[c])
                                if h == HPC - 1:
                                    fillers.extend(oproj_chunk_units(c))
                        drain(1)

            # ---- emission ----
            full_tg(0, nc.scalar)
            full_tg(1, nc.scalar)
            queue_tg(2)
            queue_tg(3)
            attention(0)
            attention(1)
            drain(len(fillers))

    nc.compile()
    return nc


_NC_CACHE = None


def _get_nc():
    global _NC_CACHE
    if _NC_CACHE is None:
        _NC_CACHE = build_kernel()
    return _NC_CACHE


_PERM = np.concatenate([np.arange(0, DK, 2), np.arange(1, DK, 2)])


def make_in_maps(in_features, token_positions, Wq, bq, Wk, bk, Wo, bo):
    import ml_dtypes

    BF = ml_dtypes.bfloat16
    x = np.ascontiguousarray(np.asarray(in_features, dtype=np.float32))
    pos = np.asarray(token_positions, dtype=np.float32)
    Wq = np.asarray(Wq, dtype=np.float32)
    bq = np.asarray(bq, dtype=np.float32)
    Wk = np.asarray(Wk, dtype=np.float32)
    bk = np.asarray(bk, dtype=np.float32)
    Wo = np.asarray(Wo, dtype=np.float32)
    bo = np.asarray(bo, dtype=np.float32)

    inv = (1.0 / THETA ** (np.arange(0, DK, 2, dtype=np.float32) / DK)).astype(
        np.float32
    )
    ang = pos[:, None] * inv[None, :]  # [T, 32]
    cos = np.cos(ang).astype(np.float32)
    sin = np.sin(ang).astype(np.float32)
    # table layout: [p, j, i] with token t = 128*j + p
    cosT = cos.reshape(NT, 128, DK // 2).transpose(1, 0, 2)
    cosT = np.ascontiguousarray(
        np.concatenate([cosT, cosT], axis=2).reshape(128, NT * DK)
    )
    sinT = np.ascontiguousarray(
        sin.reshape(NT, 128, DK // 2).transpose(1, 0, 2).reshape(128, NT * (DK // 2))
    )
    ii = np.arange(128)
    maskt = (ii[None, :] >= ii[:, None]).astype(BF)  # [k, q]: keep q >= k

    in_maps = []
    for c in range(NCORE):
        b, g = c // GPB, c % GPB
        cols = np.concatenate([DK * (HPC * g + hh) + _PERM for hh in range(HPC)])
        in_maps.append(
            {
                "xT": np.ascontiguousarray(x[b].T).astype(BF),
                "wqk": np.ascontiguousarray(
                    np.concatenate([Wq[cols].T, Wk[cols].T], axis=1)
                ).astype(BF),
                "wo": np.ascontiguousarray(Wo[:, cols].T).astype(BF),
                "bqk": np.ascontiguousarray(
                    np.concatenate([bq[cols], bk[cols]])[None, :]
                ),
                "bo": np.ascontiguousarray(
                    (bo if g == 0 else np.zeros_like(bo))[None, :]
                ),
                "cosT": cosT,
                "sinT": sinT,
                "maskt": maskt,
            }
        )
    return in_maps


def kernel(in_features, token_positions, Wq, bq, Wk, bk, Wv=None, bv=None, Wo=None, bo=None):
    from concourse import bass_utils

    nc = _get_nc()
    in_maps = make_in_maps(in_features, token_positions, Wq, bq, Wk, bk, Wo, bo)
    res = bass_utils.run_bass_kernel_spmd(
        nc,
        in_maps,
        core_ids=list(range(NCORE)),
        trace=bool(int(os.environ.get("KERNEL_TRACE", "0"))),
    )
    outs = [res.results[c]["out"] for c in range(NCORE)]
    full = np.stack(
        [np.sum(outs[b * GPB : (b + 1) * GPB], axis=0) for b in range(B)]
    ).astype(np.float32)
    kernel.last_results = res
    return full
